# revision 6
# baseline (speedup 1.0000x reference)
"""Trainium2 Bass kernel for MatrixFactorizationIF (embedding-lookup style).

Computation (per batch element b with indices i, j, k):
    pFi = pF[i]                       # [448] = [64 | 192 (Vs, r-major s-fast) | 192 (Vg)]
    out[b] = ALPHA * <pFi[:64], M[j]>
           + BETA^2 * sum_s ( <Vs[:, s], M[j]> * <Vg[:, s], M[k]> )

Strategy: data-parallel over the batch across 8 NeuronCores. pF and M are
replicated to every core's HBM; each core gathers its own rows via
SWDGE indirect DMA (one descriptor per row) and does the dot products on
the vector engine with batch elements on partitions.

Layout per tile (P=128 partitions, T elements per partition):
    element (p, t) of the tile = batch index  chunk_base + p*T + t
so the per-tile output [128, T] stores back to DRAM as one contiguous chunk.
"""

import numpy as np

# Problem constants (hardcoded per the harness contract).
N_P = 100000
N_M = 100000
R = 64
S = 3
E = R * (1 + 2 * S)  # 448
B = 500000
ALPHA = 0.001
BETA = 0.001

N_CORES = 8
P = 128
T = 20                      # batch elements per partition per tile
TILE = P * T                # batch elements per tile
BS = B // N_CORES           # 62500 batch elements per core
NTILES = -(-BS // TILE)     # 25
BS_PAD = NTILES * TILE      # 64000


def build_program(n_pf, n_m, bs_pad, t=T, repeat=1):
    import concourse.bass as bass
    import concourse.bacc as bacc
    import concourse.mybir as mybir
    from concourse.tile import TileContext

    tile_elems = P * t
    ntiles = bs_pad // tile_elems
    f32 = mybir.dt.float32
    i32 = mybir.dt.int32
    mult = mybir.AluOpType.mult
    add = mybir.AluOpType.add
    AX = mybir.AxisListType.X

    nc = bacc.Bacc("TRN2", target_bir_lowering=False)
    pF = nc.dram_tensor("pF", [n_pf, E], f32, kind="ExternalInput")
    M = nc.dram_tensor("M", [n_m, R], f32, kind="ExternalInput")
    ijk = nc.dram_tensor("ijk", [bs_pad, 3], i32, kind="ExternalInput")
    out = nc.dram_tensor("out", [bs_pad], f32, kind="ExternalOutput")

    with TileContext(nc) as tc:
        with (
            tc.tile_pool(name="idx", bufs=3) as idx_pool,
            tc.tile_pool(name="pf", bufs=2) as pf_pool,
            tc.tile_pool(name="m", bufs=2) as m_pool,
            tc.tile_pool(name="prod", bufs=3) as prod_pool,
            tc.tile_pool(name="small", bufs=3) as small_pool,
            tc.tile_pool(name="res", bufs=3) as res_pool,
        ):
            for it in range(ntiles * repeat):
                c0 = (it % ntiles) * tile_elems

                idx_t = idx_pool.tile([P, t * 3], i32)
                nc.sync.dma_start(
                    out=idx_t[:],
                    in_=ijk[c0:c0 + tile_elems, :].rearrange(
                        "(p t) c -> p (t c)", p=P),
                )
                idx3 = idx_t[:].rearrange("p (t c) -> p t c", c=3)

                pf_t = pf_pool.tile([P, t * E], f32)
                mj_t = m_pool.tile([P, t * R], f32, tag="mj")
                mk_t = m_pool.tile([P, t * R], f32, tag="mk")
                pf4 = pf_t[:].rearrange("p (t e) -> p t e", e=E)
                mj3 = mj_t[:].rearrange("p (t r) -> p t r", r=R)
                mk3 = mk_t[:].rearrange("p (t r) -> p t r", r=R)

                # HW indirect DMA consumes one offset per partition
                # descriptor: gather 128 rows per call, T calls per tile.
                for tt in range(t):
                    nc.gpsimd.indirect_dma_start(
                        out=pf4[:, tt, :], out_offset=None,
                        in_=pF[:],
                        in_offset=bass.IndirectOffsetOnAxis(
                            ap=idx3[:, tt, 0:1], axis=0),
                    )
                    nc.gpsimd.indirect_dma_start(
                        out=mj3[:, tt, :], out_offset=None,
                        in_=M[:],
                        in_offset=bass.IndirectOffsetOnAxis(
                            ap=idx3[:, tt, 1:2], axis=0),
                    )
                    nc.gpsimd.indirect_dma_start(
                        out=mk3[:, tt, :], out_offset=None,
                        in_=M[:],
                        in_offset=bass.IndirectOffsetOnAxis(
                            ap=idx3[:, tt, 2:3], axis=0),
                    )

                # dot = sum_r Pi * Mj
                prod1 = prod_pool.tile([P, t * R], f32, tag="prod1")
                nc.vector.tensor_tensor(
                    out=prod1[:].rearrange("p (t r) -> p t r", r=R),
                    in0=pf4[:, :, 0:R], in1=mj3, op=mult)
                dot = small_pool.tile([P, t], f32, tag="dot")
                nc.vector.reduce_sum(
                    out=dot[:],
                    in_=prod1[:].rearrange("p (t r) -> p t r", r=R), axis=AX)

                # a[t, s] = sum_r Vs[t, r, s] * Mj[t, r]   (unscaled by BETA)
                vs_view = pf4[:, :, R:(1 + S) * R].rearrange(
                    "p t (r s) -> p t s r", s=S)
                mjb = mj3[:, :, None, :].to_broadcast([P, t, S, R])
                pvs = prod_pool.tile([P, t * S * R], f32, tag="pvs")
                nc.vector.tensor_tensor(
                    out=pvs[:].rearrange("p (t s r) -> p t s r", s=S, r=R),
                    in0=vs_view, in1=mjb, op=mult)
                a_t = small_pool.tile([P, t * S], f32, tag="a")
                nc.vector.reduce_sum(
                    out=a_t[:],
                    in_=pvs[:].rearrange("p (t s r) -> p t s r", s=S, r=R),
                    axis=AX)

                # g[t, s] = sum_r Vg[t, r, s] * Mk[t, r]
                vg_view = pf4[:, :, (1 + S) * R:].rearrange(
                    "p t (r s) -> p t s r", s=S)
                mkb = mk3[:, :, None, :].to_broadcast([P, t, S, R])
                pvg = prod_pool.tile([P, t * S * R], f32, tag="pvg")
                nc.vector.tensor_tensor(
                    out=pvg[:].rearrange("p (t s r) -> p t s r", s=S, r=R),
                    in0=vg_view, in1=mkb, op=mult)
                g_t = small_pool.tile([P, t * S], f32, tag="g")
                nc.vector.reduce_sum(
                    out=g_t[:],
                    in_=pvg[:].rearrange("p (t s r) -> p t s r", s=S, r=R),
                    axis=AX)

                # agdot = sum_s a*g ; res = ALPHA*dot + BETA^2*agdot
                agp = small_pool.tile([P, t * S], f32, tag="agp")
                nc.vector.tensor_mul(out=agp[:], in0=a_t[:], in1=g_t[:])
                agdot = small_pool.tile([P, t], f32, tag="agdot")
                nc.vector.reduce_sum(
                    out=agdot[:],
                    in_=agp[:].rearrange("p (t s) -> p t s", s=S), axis=AX)
                agdot_b = small_pool.tile([P, t], f32, tag="agdot_b")
                nc.vector.tensor_scalar_mul(
                    out=agdot_b[:], in0=agdot[:], scalar1=BETA * BETA)
                res = res_pool.tile([P, t], f32, tag="res")
                nc.vector.scalar_tensor_tensor(
                    out=res[:], in0=dot[:], scalar=ALPHA, in1=agdot_b[:],
                    op0=mult, op1=add)

                nc.sync.dma_start(
                    out=out[c0:c0 + tile_elems].rearrange("(p t) -> p t", p=P),
                    in_=res[:])

    nc.compile()
    return nc


_NC_CACHE = {}


def _get_program():
    key = (N_P, N_M, BS_PAD, T)
    if key not in _NC_CACHE:
        _NC_CACHE[key] = build_program(N_P, N_M, BS_PAD, T)
    return _NC_CACHE[key]


def kernel(pF, M, ijk):
    from concourse.bass_utils import run_bass_kernel_spmd

    pF = np.ascontiguousarray(np.asarray(pF, dtype=np.float32))
    M = np.ascontiguousarray(np.asarray(M, dtype=np.float32))
    ijk = np.asarray(ijk)
    out_dtype = np.float32
    ijk = np.ascontiguousarray(ijk.astype(np.int32))

    nc = _get_program()

    in_maps = []
    for c in range(N_CORES):
        shard = ijk[c * BS:(c + 1) * BS]
        if shard.shape[0] < BS_PAD:
            pad = np.zeros((BS_PAD - shard.shape[0], 3), dtype=np.int32)
            shard = np.concatenate([shard, pad], axis=0)
        in_maps.append({"pF": pF, "M": M, "ijk": np.ascontiguousarray(shard)})

    results = run_bass_kernel_spmd(
        nc, in_maps, core_ids=list(range(N_CORES))).results

    outs = [results[c]["out"][:BS] for c in range(N_CORES)]
    return np.concatenate(outs).astype(out_dtype)


# revision 7
# speedup vs baseline: 1.2903x; 1.2903x over previous
"""Trainium2 Bass kernel for MatrixFactorizationIF (embedding-lookup style).

Computation (per batch element b with indices i, j, k):
    pFi = pF[i]                # [448] = [64 | 192 (Vs, r-major s-fast) | 192 (Vg)]
    out[b] = ALPHA * <pFi[:64], M[j]>
           + BETA^2 * sum_s ( <Vs[:, s], M[j]> * <Vg[:, s], M[k]> )

Strategy: the fast gather primitive (InstDMAGatherAnt) takes int16 row
indices, so the 100k-row tables are addressed through range/stride tricks:

  - Shard the batch across 8 cores by i-range: core c serves i in
    [12500c, 12500(c+1)) and holds only that pF slice (i_loc < 12500).
  - Within a core, sort elements into 32 groups by (j%4, k%8). For group
    (jc, kc), M[j] rows are gathered from the strided view M[jc::4]
    (row stride 1024B, local index j>>2 < 25000) and M[k] rows from
    M[kc::8] (stride 2048B, k>>3 < 12500) -- all int16-safe.
  - Groups are padded to a fixed 2176 elements (mean 1953, sigma ~44; the
    static program needs fixed group sizes). Gather calls are capped at
    1024 indices (HW limit: 64 descriptors per Q7 lane), so each group
    does 3 calls per gathered tensor: 1024 + 1024 + 128 rows.
  - All index preparation / permutation happens host-side in kernel();
    outputs are scattered back to the original order on the host.

Per-group tile (P=128 partitions, T=17 elements per partition): gather
element e lands at (p=e%128, t=e//128); compute is batch-on-partitions
vector work; the [128, 17] result stores to DRAM as one contiguous chunk.
"""

import numpy as np

# Problem constants (hardcoded per the harness contract).
N_P = 100000
N_M = 100000
R = 64
S = 3
E = R * (1 + 2 * S)  # 448
B = 500000
ALPHA = 0.001
BETA = 0.001

N_CORES = 8
P = 128
PF_SHARD = N_P // N_CORES   # 12500 pF rows per core
NJ = 4                      # j stride classes
NK = 8                      # k stride classes
NG = NJ * NK                # 32 groups per core
GCAP = 2176                 # padded group capacity (17 * 128)
GCOLS = GCAP // 16          # 136 wrapped-index columns
TCOL = GCAP // P            # 17
BS_PAD = NG * GCAP          # 69632 padded elements per core
MAX_GATHER = 1024           # HW: <= 64 descriptors per Q7 lane
GATHER_CALLS = [(0, 1024), (1024, 1024), (2048, 128)]  # (elem offset, num)


def build_program(repeat=1):
    import concourse.bass as bass
    import concourse.bacc as bacc
    import concourse.mybir as mybir
    from concourse.tile import TileContext

    f32 = mybir.dt.float32
    i16 = mybir.dt.int16
    mult = mybir.AluOpType.mult
    add = mybir.AluOpType.add
    AX = mybir.AxisListType.X
    t = TCOL

    nc = bacc.Bacc("TRN2", target_bir_lowering=False)
    pFs = nc.dram_tensor("pFs", [PF_SHARD, E], f32, kind="ExternalInput")
    M = nc.dram_tensor("M", [N_M, R], f32, kind="ExternalInput")
    idx = nc.dram_tensor("idx", [NG, P, 3 * GCOLS], i16, kind="ExternalInput")
    out = nc.dram_tensor("out", [BS_PAD], f32, kind="ExternalOutput")

    with TileContext(nc) as tc:
        with (
            tc.tile_pool(name="idx", bufs=3) as idx_pool,
            tc.tile_pool(name="pf", bufs=2) as pf_pool,
            tc.tile_pool(name="m", bufs=2) as m_pool,
            tc.tile_pool(name="prod", bufs=3) as prod_pool,
            tc.tile_pool(name="small", bufs=3) as small_pool,
            tc.tile_pool(name="res", bufs=3) as res_pool,
        ):
            for it in range(NG * repeat):
                g = it % NG
                jc, kc = g // NK, g % NK
                c0 = g * GCAP

                idx_t = idx_pool.tile([P, 3 * GCOLS], i16)
                nc.sync.dma_start(out=idx_t[:], in_=idx[g])

                pf_t = pf_pool.tile([P, t * E], f32)
                mj_t = m_pool.tile([P, t * R], f32, tag="mj")
                mk_t = m_pool.tile([P, t * R], f32, tag="mk")
                pf4 = pf_t[:].rearrange("p (t e) -> p t e", e=E)
                mj3 = mj_t[:].rearrange("p (t r) -> p t r", r=R)
                mk3 = mk_t[:].rearrange("p (t r) -> p t r", r=R)

                mjview = M[:].rearrange(
                    "(n f) r -> n (f r)", f=NJ)[:, jc * R:(jc + 1) * R]
                mkview = M[:].rearrange(
                    "(n f) r -> n (f r)", f=NK)[:, kc * R:(kc + 1) * R]

                for eo, num in GATHER_CALLS:
                    ic0, icn = eo // 16, num // 16
                    oc0, ocn = eo // P, num // P
                    nc.gpsimd.dma_gather(
                        out_ap=pf4[:, oc0:oc0 + ocn, :],
                        in_ap=pFs[:],
                        idxs_ap=idx_t[:, ic0:ic0 + icn],
                        num_idxs=num, num_idxs_reg=num, elem_size=E)
                    nc.gpsimd.dma_gather(
                        out_ap=mj3[:, oc0:oc0 + ocn, :],
                        in_ap=mjview,
                        idxs_ap=idx_t[:, GCOLS + ic0:GCOLS + ic0 + icn],
                        num_idxs=num, num_idxs_reg=num, elem_size=R,
                        elem_step=R * NJ)
                    nc.gpsimd.dma_gather(
                        out_ap=mk3[:, oc0:oc0 + ocn, :],
                        in_ap=mkview,
                        idxs_ap=idx_t[:, 2 * GCOLS + ic0:2 * GCOLS + ic0 + icn],
                        num_idxs=num, num_idxs_reg=num, elem_size=R,
                        elem_step=R * NK)

                # dot = sum_r Pi * Mj
                prod1 = prod_pool.tile([P, t * R], f32, tag="prod1")
                nc.vector.tensor_tensor(
                    out=prod1[:].rearrange("p (t r) -> p t r", r=R),
                    in0=pf4[:, :, 0:R], in1=mj3, op=mult)
                dot = small_pool.tile([P, t], f32, tag="dot")
                nc.vector.reduce_sum(
                    out=dot[:],
                    in_=prod1[:].rearrange("p (t r) -> p t r", r=R), axis=AX)

                # a[t, s] = sum_r Vs[t, r, s] * Mj[t, r]   (unscaled by BETA)
                vs_view = pf4[:, :, R:(1 + S) * R].rearrange(
                    "p t (r s) -> p t s r", s=S)
                mjb = mj3[:, :, None, :].to_broadcast([P, t, S, R])
                pvs = prod_pool.tile([P, t * S * R], f32, tag="pvs")
                nc.vector.tensor_tensor(
                    out=pvs[:].rearrange("p (t s r) -> p t s r", s=S, r=R),
                    in0=vs_view, in1=mjb, op=mult)
                a_t = small_pool.tile([P, t * S], f32, tag="a")
                nc.vector.reduce_sum(
                    out=a_t[:],
                    in_=pvs[:].rearrange("p (t s r) -> p t s r", s=S, r=R),
                    axis=AX)

                # g[t, s] = sum_r Vg[t, r, s] * Mk[t, r]
                vg_view = pf4[:, :, (1 + S) * R:].rearrange(
                    "p t (r s) -> p t s r", s=S)
                mkb = mk3[:, :, None, :].to_broadcast([P, t, S, R])
                pvg = prod_pool.tile([P, t * S * R], f32, tag="pvg")
                nc.vector.tensor_tensor(
                    out=pvg[:].rearrange("p (t s r) -> p t s r", s=S, r=R),
                    in0=vg_view, in1=mkb, op=mult)
                g_t = small_pool.tile([P, t * S], f32, tag="g")
                nc.vector.reduce_sum(
                    out=g_t[:],
                    in_=pvg[:].rearrange("p (t s r) -> p t s r", s=S, r=R),
                    axis=AX)

                # agdot = sum_s a*g ; res = ALPHA*dot + BETA^2*agdot
                agp = small_pool.tile([P, t * S], f32, tag="agp")
                nc.vector.tensor_mul(out=agp[:], in0=a_t[:], in1=g_t[:])
                agdot = small_pool.tile([P, t], f32, tag="agdot")
                nc.vector.reduce_sum(
                    out=agdot[:],
                    in_=agp[:].rearrange("p (t s) -> p t s", s=S), axis=AX)
                agdot_b = small_pool.tile([P, t], f32, tag="agdot_b")
                nc.vector.tensor_scalar_mul(
                    out=agdot_b[:], in0=agdot[:], scalar1=BETA * BETA)
                res = res_pool.tile([P, t], f32, tag="res")
                nc.vector.scalar_tensor_tensor(
                    out=res[:], in0=dot[:], scalar=ALPHA, in1=agdot_b[:],
                    op0=mult, op1=add)

                # element e of the group sits at (p=e%128, t=e//128)
                nc.sync.dma_start(
                    out=out[c0:c0 + GCAP].rearrange("(t p) -> p t", p=P),
                    in_=res[:])

    nc.compile()
    return nc


_NC_CACHE = {}


def _get_program():
    if "main" not in _NC_CACHE:
        _NC_CACHE["main"] = build_program()
    return _NC_CACHE["main"]


def prepare_inputs(pF, M, ijk):
    """Host-side shard + sort + pad. Returns (in_maps, src_index) where
    src_index[b] is the flat position of original element b in the
    concatenated per-core padded outputs."""
    i = ijk[:, 0].astype(np.int64)
    j = ijk[:, 1].astype(np.int64)
    k = ijk[:, 2].astype(np.int64)

    core = i // PF_SHARD
    gl = (j % NJ) * NK + (k % NK)            # group within core
    gg = core * NG + gl                      # global group id, 0..255
    order = np.argsort(gg, kind="stable")
    counts = np.bincount(gg, minlength=N_CORES * NG)
    if counts.max() > GCAP:
        raise RuntimeError(
            f"group overflow: max {counts.max()} > {GCAP}; input index "
            f"distribution too skewed for the static schedule")
    starts = np.zeros(N_CORES * NG, np.int64)
    starts[1:] = np.cumsum(counts)[:-1]
    rank = np.arange(B) - np.repeat(starts, counts)   # rank within group,
    # in sorted order; map back to original element positions:
    rank_orig = np.empty(B, np.int64)
    rank_orig[order] = rank
    src_index = core * BS_PAD + gl * GCAP + rank_orig

    i_loc = (i - core * PF_SHARD).astype(np.int16)
    j_loc = (j >> 2).astype(np.int16)
    k_loc = (k >> 3).astype(np.int16)

    # wrapped idx layout: element rank e -> [e % 16, e // 16]
    wrapped = np.zeros((N_CORES, NG, 3, 16, GCOLS), np.int16)
    wp = (rank_orig % 16).astype(np.int64)
    ws = (rank_orig // 16).astype(np.int64)
    wrapped[core, gl, 0, wp, ws] = i_loc
    wrapped[core, gl, 1, wp, ws] = j_loc
    wrapped[core, gl, 2, wp, ws] = k_loc
    # replicate the 16-partition wrap to all 128 partitions; free-dim
    # layout per partition is [tensor, col] -> 3*GCOLS
    wrapped = np.tile(wrapped, (1, 1, 1, 8, 1))               # [.., 128, GCOLS]
    wrapped = wrapped.transpose(0, 1, 3, 2, 4).reshape(
        N_CORES, NG, P, 3 * GCOLS)

    in_maps = []
    for c in range(N_CORES):
        in_maps.append({
            "pFs": np.ascontiguousarray(pF[c * PF_SHARD:(c + 1) * PF_SHARD]),
            "M": M,
            "idx": np.ascontiguousarray(wrapped[c]),
        })
    return in_maps, src_index


def kernel(pF, M, ijk):
    from concourse.bass_utils import run_bass_kernel_spmd

    pF = np.ascontiguousarray(np.asarray(pF, dtype=np.float32))
    M = np.ascontiguousarray(np.asarray(M, dtype=np.float32))
    ijk = np.asarray(ijk)

    nc = _get_program()
    in_maps, src_index = prepare_inputs(pF, M, ijk)

    results = run_bass_kernel_spmd(
        nc, in_maps, core_ids=list(range(N_CORES))).results

    flat = np.concatenate([results[c]["out"] for c in range(N_CORES)])
    return flat[src_index].astype(np.float32)


# revision 8
# speedup vs baseline: 2.1758x; 1.6862x over previous
"""Trainium2 Bass kernel for MatrixFactorizationIF (embedding-lookup style).

Computation (per batch element b with indices i, j, k):
    pFi = pF[i]                # [448] = [64 | 192 (Vs, r-major s-fast) | 192 (Vg)]
    out[b] = ALPHA * <pFi[:64], M[j]>
           + BETA^2 * sum_s ( <Vs[:, s], M[j]> * <Vg[:, s], M[k]> )

Strategy: the fast gather primitive (InstDMAGatherAnt) takes int16 row
indices, so the 100k-row tables are addressed through range/stride tricks:

  - Shard the batch across 8 cores by i-range: core c serves i in
    [12500c, 12500(c+1)) and holds only that pF slice (i_loc < 12500).
  - Within a core, sort elements into 32 groups by (j%4, k%8). For group
    (jc, kc), M[j] rows are gathered from the strided view M[jc::4]
    (row stride 1024B, local index j>>2 < 25000) and M[k] rows from
    M[kc::8] (stride 2048B, k>>3 < 12500) -- all int16-safe.
  - Groups are padded to a fixed 2176 elements (mean 1953, sigma ~44; the
    static program needs fixed group sizes). Gather calls are capped at
    1024 indices (HW limit: 64 descriptors per Q7 lane), so each group
    does 3 calls per gathered tensor: 1024 + 1024 + 128 rows.
  - All index preparation / permutation happens host-side in kernel();
    outputs are scattered back to the original order on the host.

Per-group tile (P=128 partitions, T=17 elements per partition): gather
element e lands at (p=e%128, t=e//128); compute is batch-on-partitions
vector work; the [128, 17] result stores to DRAM as one contiguous chunk.
"""

import numpy as np

# Problem constants (hardcoded per the harness contract).
N_P = 100000
N_M = 100000
R = 64
S = 3
E = R * (1 + 2 * S)  # 448
B = 500000
ALPHA = 0.001
BETA = 0.001

N_CORES = 8
P = 128
PF_SHARD = N_P // N_CORES   # 12500 pF rows per core
NJ = 4                      # j stride classes
NK = 8                      # k stride classes
NG = NJ * NK                # 32 groups per core
GCAP = 2176                 # padded group capacity (17 * 128)
GCOLS = GCAP // 16          # 136 wrapped-index columns
TCOL = GCAP // P            # 17
BS_PAD = NG * GCAP          # 69632 padded elements per core
MAX_GATHER = 1024           # HW: <= 64 descriptors per Q7 lane
GATHER_CALLS = [(0, 1024), (1024, 1024), (2048, 128)]  # (elem offset, num)


def build_program(repeat=1):
    import concourse.bass as bass
    import concourse.bacc as bacc
    import concourse.mybir as mybir
    from concourse.tile import TileContext

    f32 = mybir.dt.float32
    i16 = mybir.dt.int16
    mult = mybir.AluOpType.mult
    add = mybir.AluOpType.add
    AX = mybir.AxisListType.X
    t = TCOL

    nc = bacc.Bacc("TRN2", target_bir_lowering=False, num_swdge_queues=3)
    pFs = nc.dram_tensor("pFs", [PF_SHARD, E], f32, kind="ExternalInput")
    M = nc.dram_tensor("M", [N_M, R], f32, kind="ExternalInput")
    idx = nc.dram_tensor("idx", [NG, P, 3 * GCOLS], i16, kind="ExternalInput")
    out = nc.dram_tensor("out", [BS_PAD], f32, kind="ExternalOutput")

    with TileContext(nc) as tc:
        with (
            tc.tile_pool(name="idx", bufs=3) as idx_pool,
            tc.tile_pool(name="pf", bufs=2) as pf_pool,
            tc.tile_pool(name="m", bufs=2) as m_pool,
            tc.tile_pool(name="prod", bufs=3) as prod_pool,
            tc.tile_pool(name="small", bufs=3) as small_pool,
            tc.tile_pool(name="res", bufs=3) as res_pool,
        ):
            for it in range(NG * repeat):
                g = it % NG
                jc, kc = g // NK, g % NK
                c0 = g * GCAP

                idx_t = idx_pool.tile([P, 3 * GCOLS], i16)
                nc.sync.dma_start(out=idx_t[:], in_=idx[g])

                pf_t = pf_pool.tile([P, t * E], f32)
                mj_t = m_pool.tile([P, t * R], f32, tag="mj")
                mk_t = m_pool.tile([P, t * R], f32, tag="mk")
                pf4 = pf_t[:].rearrange("p (t e) -> p t e", e=E)
                mj3 = mj_t[:].rearrange("p (t r) -> p t r", r=R)
                mk3 = mk_t[:].rearrange("p (t r) -> p t r", r=R)

                mjview = M[:].rearrange(
                    "(n f) r -> n (f r)", f=NJ)[:, jc * R:(jc + 1) * R]
                mkview = M[:].rearrange(
                    "(n f) r -> n (f r)", f=NK)[:, kc * R:(kc + 1) * R]

                for eo, num in GATHER_CALLS:
                    ic0, icn = eo // 16, num // 16
                    oc0, ocn = eo // P, num // P
                    nc.gpsimd.dma_gather(
                        out_ap=pf4[:, oc0:oc0 + ocn, :],
                        in_ap=pFs[:],
                        idxs_ap=idx_t[:, ic0:ic0 + icn],
                        num_idxs=num, num_idxs_reg=num, elem_size=E,
                        queue_num=0)
                    nc.gpsimd.dma_gather(
                        out_ap=mj3[:, oc0:oc0 + ocn, :],
                        in_ap=mjview,
                        idxs_ap=idx_t[:, GCOLS + ic0:GCOLS + ic0 + icn],
                        num_idxs=num, num_idxs_reg=num, elem_size=R,
                        elem_step=R * NJ, queue_num=1)
                    nc.gpsimd.dma_gather(
                        out_ap=mk3[:, oc0:oc0 + ocn, :],
                        in_ap=mkview,
                        idxs_ap=idx_t[:, 2 * GCOLS + ic0:2 * GCOLS + ic0 + icn],
                        num_idxs=num, num_idxs_reg=num, elem_size=R,
                        elem_step=R * NK, queue_num=2)

                # dot = sum_r Pi * Mj
                prod1 = prod_pool.tile([P, t * R], f32, tag="prod1")
                nc.vector.tensor_tensor(
                    out=prod1[:].rearrange("p (t r) -> p t r", r=R),
                    in0=pf4[:, :, 0:R], in1=mj3, op=mult)
                dot = small_pool.tile([P, t], f32, tag="dot")
                nc.vector.reduce_sum(
                    out=dot[:],
                    in_=prod1[:].rearrange("p (t r) -> p t r", r=R), axis=AX)

                # a[t, s] = sum_r Vs[t, r, s] * Mj[t, r]   (unscaled by BETA)
                vs_view = pf4[:, :, R:(1 + S) * R].rearrange(
                    "p t (r s) -> p t s r", s=S)
                mjb = mj3[:, :, None, :].to_broadcast([P, t, S, R])
                pvs = prod_pool.tile([P, t * S * R], f32, tag="pvs")
                nc.vector.tensor_tensor(
                    out=pvs[:].rearrange("p (t s r) -> p t s r", s=S, r=R),
                    in0=vs_view, in1=mjb, op=mult)
                a_t = small_pool.tile([P, t * S], f32, tag="a")
                nc.vector.reduce_sum(
                    out=a_t[:],
                    in_=pvs[:].rearrange("p (t s r) -> p t s r", s=S, r=R),
                    axis=AX)

                # g[t, s] = sum_r Vg[t, r, s] * Mk[t, r]
                vg_view = pf4[:, :, (1 + S) * R:].rearrange(
                    "p t (r s) -> p t s r", s=S)
                mkb = mk3[:, :, None, :].to_broadcast([P, t, S, R])
                pvg = prod_pool.tile([P, t * S * R], f32, tag="pvg")
                nc.vector.tensor_tensor(
                    out=pvg[:].rearrange("p (t s r) -> p t s r", s=S, r=R),
                    in0=vg_view, in1=mkb, op=mult)
                g_t = small_pool.tile([P, t * S], f32, tag="g")
                nc.vector.reduce_sum(
                    out=g_t[:],
                    in_=pvg[:].rearrange("p (t s r) -> p t s r", s=S, r=R),
                    axis=AX)

                # agdot = sum_s a*g ; res = ALPHA*dot + BETA^2*agdot
                agp = small_pool.tile([P, t * S], f32, tag="agp")
                nc.vector.tensor_mul(out=agp[:], in0=a_t[:], in1=g_t[:])
                agdot = small_pool.tile([P, t], f32, tag="agdot")
                nc.vector.reduce_sum(
                    out=agdot[:],
                    in_=agp[:].rearrange("p (t s) -> p t s", s=S), axis=AX)
                agdot_b = small_pool.tile([P, t], f32, tag="agdot_b")
                nc.vector.tensor_scalar_mul(
                    out=agdot_b[:], in0=agdot[:], scalar1=BETA * BETA)
                res = res_pool.tile([P, t], f32, tag="res")
                nc.vector.scalar_tensor_tensor(
                    out=res[:], in0=dot[:], scalar=ALPHA, in1=agdot_b[:],
                    op0=mult, op1=add)

                # element e of the group sits at (p=e%128, t=e//128)
                nc.sync.dma_start(
                    out=out[c0:c0 + GCAP].rearrange("(t p) -> p t", p=P),
                    in_=res[:])

    nc.compile()
    return nc


_NC_CACHE = {}


def _get_program():
    if "main" not in _NC_CACHE:
        _NC_CACHE["main"] = build_program()
    return _NC_CACHE["main"]


def prepare_inputs(pF, M, ijk):
    """Host-side shard + sort + pad. Returns (in_maps, src_index) where
    src_index[b] is the flat position of original element b in the
    concatenated per-core padded outputs."""
    i = ijk[:, 0].astype(np.int64)
    j = ijk[:, 1].astype(np.int64)
    k = ijk[:, 2].astype(np.int64)

    core = i // PF_SHARD
    gl = (j % NJ) * NK + (k % NK)            # group within core
    gg = core * NG + gl                      # global group id, 0..255
    i_loc_key = i - core * PF_SHARD
    order = np.argsort(gg * 16384 + i_loc_key, kind="stable")
    counts = np.bincount(gg, minlength=N_CORES * NG)
    if counts.max() > GCAP:
        raise RuntimeError(
            f"group overflow: max {counts.max()} > {GCAP}; input index "
            f"distribution too skewed for the static schedule")
    starts = np.zeros(N_CORES * NG, np.int64)
    starts[1:] = np.cumsum(counts)[:-1]
    rank = np.arange(B) - np.repeat(starts, counts)   # rank within group,
    # in sorted order; map back to original element positions:
    rank_orig = np.empty(B, np.int64)
    rank_orig[order] = rank
    src_index = core * BS_PAD + gl * GCAP + rank_orig

    i_loc = (i - core * PF_SHARD).astype(np.int16)
    j_loc = (j >> 2).astype(np.int16)
    k_loc = (k >> 3).astype(np.int16)

    # wrapped idx layout: element rank e -> [e % 16, e // 16]
    wrapped = np.zeros((N_CORES, NG, 3, 16, GCOLS), np.int16)
    wp = (rank_orig % 16).astype(np.int64)
    ws = (rank_orig // 16).astype(np.int64)
    wrapped[core, gl, 0, wp, ws] = i_loc
    wrapped[core, gl, 1, wp, ws] = j_loc
    wrapped[core, gl, 2, wp, ws] = k_loc
    # replicate the 16-partition wrap to all 128 partitions; free-dim
    # layout per partition is [tensor, col] -> 3*GCOLS
    wrapped = np.tile(wrapped, (1, 1, 1, 8, 1))               # [.., 128, GCOLS]
    wrapped = wrapped.transpose(0, 1, 3, 2, 4).reshape(
        N_CORES, NG, P, 3 * GCOLS)

    in_maps = []
    for c in range(N_CORES):
        in_maps.append({
            "pFs": np.ascontiguousarray(pF[c * PF_SHARD:(c + 1) * PF_SHARD]),
            "M": M,
            "idx": np.ascontiguousarray(wrapped[c]),
        })
    return in_maps, src_index


def kernel(pF, M, ijk):
    from concourse.bass_utils import run_bass_kernel_spmd

    pF = np.ascontiguousarray(np.asarray(pF, dtype=np.float32))
    M = np.ascontiguousarray(np.asarray(M, dtype=np.float32))
    ijk = np.asarray(ijk)

    nc = _get_program()
    in_maps, src_index = prepare_inputs(pF, M, ijk)

    results = run_bass_kernel_spmd(
        nc, in_maps, core_ids=list(range(N_CORES))).results

    flat = np.concatenate([results[c]["out"] for c in range(N_CORES)])
    return flat[src_index].astype(np.float32)
